# revision 2
# baseline (speedup 1.0000x reference)
"""Trainium2 Bass kernel: GCN message passing (nn_DDI_gcn), 8 NeuronCores SPMD.

Math:
  agg[r] = sum_{e: row_idx[e]==r} vals[e] * mEmbed[col_idx[e] % 50000]
  out[i] = 2*(inter*relu(agg[i]) + (1-inter)*relu(agg[i+50000])),  i < 50000

Strategy (destination sharding, no cross-core reduction):
  * Core k owns output rows [6272k, 6272(k+1)); host buckets every edge by
    (core, 128-row dest tile, table half, plane) and pads each bucket run to
    a 128-edge chunk boundary.
  * Device, per dest tile ("group"): dma_gather the edges' embedding rows
    (fp16, 256 B/row) into SBUF; for each 128-edge chunk build the selection
    matrix S[e, r] = val[e]*(d[e]==r) with ONE dual-op tensor_scalar
    (is_equal -> mult) from a constant iota tile; TensorE accumulates
    S^T @ G into a per-plane PSUM tile (the segment sum); epilogue applies
    a*relu(psumA) + b*relu(psumB) and streams the 128x128 f32 tile out.
  * All index math is host-side numpy; the device never touches raw indices
    except as dma_gather int16 offsets.
"""

import numpy as np

import concourse.bass as bass
import concourse.bacc as bacc
import concourse.tile as tile
import concourse.mybir as mybir
from concourse.bass_utils import run_bass_kernel_spmd

MED = 50000
NCORES = 8
TILES = 49               # dest tiles per plane per core
RPC = TILES * 128        # 6272 dest rows per core (per plane)
HALF = 32768             # table split so gather indices fit int16
P = 128
F = 128                  # feature dim

_NC_CACHE = {}


def build_nc(C0, C1, tiles=TILES, gbufs=3, repeat=1):
    """C0/C1: chunks per half0/half1 run. Group chunk layout: [A0|B0|A1|B1].
    repeat>1 re-runs the whole body (timing: marginal = pure HW time)."""
    CG = 2 * C0 + 2 * C1
    N0 = 2 * C0 * 128    # idxs in gather call 0 (table half 0)
    N1 = 2 * C1 * 128
    IC = (N0 + N1) // 16
    dt16 = mybir.dt.float16
    f32 = mybir.dt.float32

    nc = bacc.Bacc(None, target_bir_lowering=False)
    table = nc.dram_tensor("table", [MED, F], dt16, kind="ExternalInput")
    idx_d = nc.dram_tensor("idx", [tiles, P, IC], mybir.dt.int16, kind="ExternalInput")
    dval_d = nc.dram_tensor("dval", [tiles, P, 2 * CG], f32, kind="ExternalInput")
    ab_d = nc.dram_tensor("ab", [P, 2], f32, kind="ExternalInput")
    iota_d = nc.dram_tensor("iota", [P, P], dt16, kind="ExternalInput")
    out_d = nc.dram_tensor("out", [tiles, P, F], f32, kind="ExternalOutput")

    planes = [0] * C0 + [1] * C0 + [0] * C1 + [1] * C1
    firstA, lastA = 0, 2 * C0 + C1 - 1
    firstB, lastB = C0, CG - 1

    with tile.TileContext(nc) as tc:
        with (
            tc.tile_pool(name="const", bufs=1) as constp,
            tc.tile_pool(name="gbuf", bufs=gbufs) as gbufp,
            tc.tile_pool(name="meta", bufs=3) as metap,
            tc.tile_pool(name="sp", bufs=8) as sp,
            tc.tile_pool(name="ep", bufs=4) as ep,
            tc.tile_pool(name="psum", bufs=2, space=bass.MemorySpace.PSUM) as psp,
        ):
            iota_t = constp.tile([P, P], dt16, tag="iota")
            nc.sync.dma_start(iota_t[:], iota_d[:])
            ab_t = constp.tile([P, 2], f32, tag="ab")
            nc.sync.dma_start(ab_t[:], ab_d[:])

            for g in [g_ for _ in range(repeat) for g_ in range(tiles)]:
                idx_t = metap.tile([P, IC], mybir.dt.int16, tag="idx")
                nc.sync.dma_start(idx_t[:], idx_d[g])
                dv_t = metap.tile([P, 2 * CG], f32, tag="dval")
                nc.sync.dma_start(dv_t[:], dval_d[g])

                g_t = gbufp.tile([P, CG, F], dt16, tag="g")
                nc.gpsimd.dma_gather(
                    g_t[:, 0 : 2 * C0, :], table[0:HALF, :],
                    idx_t[:, 0 : N0 // 16], N0, N0, F,
                    single_packet=False,
                )
                nc.gpsimd.dma_gather(
                    g_t[:, 2 * C0 : CG, :], table[HALF:MED, :],
                    idx_t[:, N0 // 16 : IC], N1, N1, F,
                    single_packet=False,
                )

                psA = psp.tile([P, F], f32, tag="psA")
                psB = psp.tile([P, F], f32, tag="psB")
                for c in range(CG):
                    s_t = sp.tile([P, P], dt16, tag="s")
                    nc.vector.tensor_scalar(
                        s_t[:], iota_t[:],
                        dv_t[:, c : c + 1], dv_t[:, CG + c : CG + c + 1],
                        mybir.AluOpType.is_equal, mybir.AluOpType.mult,
                    )
                    if planes[c] == 0:
                        nc.tensor.matmul(psA[:], s_t[:], g_t[:, c, :],
                                         start=(c == firstA), stop=(c == lastA))
                    else:
                        nc.tensor.matmul(psB[:], s_t[:], g_t[:, c, :],
                                         start=(c == firstB), stop=(c == lastB))

                t0 = ep.tile([P, F], f32, tag="t0")
                nc.vector.tensor_scalar(t0[:], psA[:], 0.0, ab_t[:, 0:1],
                                        mybir.AluOpType.max, mybir.AluOpType.mult)
                t1 = ep.tile([P, F], f32, tag="t1")
                nc.vector.tensor_scalar(t1[:], psB[:], 0.0, ab_t[:, 1:2],
                                        mybir.AluOpType.max, mybir.AluOpType.mult)
                o_t = ep.tile([P, F], f32, tag="o")
                nc.vector.tensor_tensor(o_t[:], t0[:], t1[:], mybir.AluOpType.add)
                nc.sync.dma_start(out_d[g], o_t[:])

    nc.compile()
    return nc


def preprocess(vals, mEmbed, inter, row_idx, col_idx, tiles=TILES):
    E = row_idx.shape[0]
    col = col_idx.astype(np.int64) % MED
    rowl = row_idx.astype(np.int64)
    plane = rowl // MED
    prow = rowl % MED
    core = np.minimum(prow // RPC, NCORES - 1)
    lt = (prow - core * RPC) >> 7
    d = (prow & 127).astype(np.float32)
    half = (col >= HALF).astype(np.int64)
    lidx = (col - half * HALF).astype(np.int16)

    run = half * 2 + plane                      # A0,B0,A1,B1 order
    key = (core * tiles + lt) * 4 + run
    order = np.argsort(key, kind="stable")
    ksort = key[order]
    nk = NCORES * tiles * 4
    cnt = np.bincount(ksort, minlength=nk)
    starts = np.concatenate([[0], np.cumsum(cnt)[:-1]])
    rank = np.arange(E, dtype=np.int64) - starts[ksort]

    cnt4 = cnt.reshape(-1, 4)
    C0 = max(1, int(np.ceil(cnt4[:, 0:2].max() / 128)))
    C1 = max(1, int(np.ceil(cnt4[:, 2:4].max() / 128)))
    CG = 2 * C0 + 2 * C1
    N0 = 2 * C0 * 128
    N1 = 2 * C1 * 128
    run_off = np.array([0, C0 * 128, 2 * C0 * 128, (2 * C0 + C1) * 128])
    SLOTS_G = CG * 128
    gidx = ksort // 4
    slot = gidx * SLOTS_G + run_off[ksort % 4] + rank
    TOT = NCORES * tiles * SLOTS_G

    IDX = np.zeros(TOT, np.int16)
    VAL = np.zeros(TOT, np.float32)
    DD = np.zeros(TOT, np.float32)
    IDX[slot] = lidx[order]
    VAL[slot] = np.asarray(vals, np.float32)[order]
    DD[slot] = d[order]

    IDX4 = IDX.reshape(NCORES, tiles, CG, 128)
    i0 = (IDX4[:, :, : 2 * C0, :].reshape(NCORES, tiles, N0 // 16, 16)
          .transpose(0, 1, 3, 2))
    i1 = (IDX4[:, :, 2 * C0 :, :].reshape(NCORES, tiles, N1 // 16, 16)
          .transpose(0, 1, 3, 2))
    idx16 = np.concatenate([i0, i1], axis=3)           # [NC, tiles, 16, IC]
    idx128 = np.ascontiguousarray(np.tile(idx16, (1, 1, 8, 1)))

    D4 = DD.reshape(NCORES, tiles, CG, 128).transpose(0, 1, 3, 2)
    V4 = VAL.reshape(NCORES, tiles, CG, 128).transpose(0, 1, 3, 2)
    dval = np.ascontiguousarray(np.concatenate([D4, V4], axis=3), dtype=np.float32)

    table16 = np.asarray(mEmbed, np.float32).astype(np.float16)
    iota = np.ascontiguousarray(
        np.broadcast_to(np.arange(128, dtype=np.float16), (128, 128)))
    a = 2.0 * np.float32(np.asarray(inter).reshape(-1)[0])
    b = np.float32(2.0) - a
    ab = np.ascontiguousarray(
        np.stack([np.full(128, a, np.float32), np.full(128, b, np.float32)], axis=1))
    return C0, C1, table16, iota, ab, idx128, dval


def _run(vals, mEmbed, inter, row_idx, col_idx, trace=False):
    C0, C1, table16, iota, ab, idx128, dval = preprocess(
        vals, mEmbed, inter, row_idx, col_idx)
    key = (C0, C1, 1, ())
    if key not in _NC_CACHE:
        _NC_CACHE[key] = build_nc(C0, C1)
    nc = _NC_CACHE[key]
    in_maps = [
        {"table": table16, "iota": iota, "ab": ab,
         "idx": idx128[k], "dval": dval[k]}
        for k in range(NCORES)
    ]
    res = run_bass_kernel_spmd(nc, in_maps, core_ids=list(range(NCORES)),
                               trace=trace)
    full = np.concatenate(
        [res.results[k]["out"].reshape(RPC, F) for k in range(NCORES)], axis=0)
    return np.ascontiguousarray(full[:MED]), res


def kernel(vals, mEmbed, inter, row_idx, col_idx):
    out, _ = _run(vals, mEmbed, inter, row_idx, col_idx, trace=False)
    return out


def _make_sharded(nc, donate=False):
    """Replicate bass2jax.run_bass_via_pjrt's executable construction so we
    can reuse it for repeated timed executions."""
    import jax
    from jax.sharding import Mesh, PartitionSpec
    from jax.experimental.shard_map import shard_map
    from concourse import bass2jax as b2j

    b2j.install_neuronx_cc_hook()
    partition_name = nc.partition_id_tensor.name if nc.partition_id_tensor else None
    in_names, out_names, out_avals, zero_outs = [], [], [], []
    for alloc in nc.m.functions[0].allocations:
        if not isinstance(alloc, mybir.MemoryLocationSet):
            continue
        name = alloc.memorylocations[0].name
        if alloc.kind == "ExternalInput":
            if name != partition_name:
                in_names.append(name)
        elif alloc.kind == "ExternalOutput":
            out_names.append(name)
            shape = tuple(alloc.tensor_shape)
            dtype = mybir.dt.np(alloc.dtype)
            out_avals.append(jax.core.ShapedArray(shape, dtype))
            zero_outs.append(np.zeros(shape, dtype))
    n_params = len(in_names)
    in_names = in_names + out_names
    if partition_name is not None:
        in_names = in_names + [partition_name]

    def _body(*args):
        operands = list(args)
        if partition_name is not None:
            operands.append(b2j.partition_id_tensor())
        outs = b2j._bass_exec_p.bind(
            *operands,
            out_avals=tuple(out_avals),
            in_names=tuple(in_names),
            out_names=tuple(out_names),
            lowering_input_output_aliases=(),
            sim_require_finite=True,
            sim_require_nnan=True,
            nc=nc,
        )
        return tuple(outs)

    devices = jax.devices()[:NCORES]
    mesh = Mesh(np.asarray(devices), ("core",))
    in_specs = (PartitionSpec("core"),) * (n_params + len(out_names))
    out_specs = (PartitionSpec("core"),) * len(out_names)
    kw = dict(donate_argnums=tuple(range(n_params, n_params + len(out_names)))) if donate else {}

    sharded = jax.jit(
        shard_map(_body, mesh=mesh, in_specs=in_specs,
                  out_specs=out_specs, check_rep=False),
        keep_unused=True, **kw)
    return sharded, mesh, in_names[:n_params], out_names, zero_outs


def timed_run(vals, mEmbed, inter, row_idx, col_idx, k=5, samples=12,
              build_kwargs=None):
    """Time on device: build the same program with the body repeated 1x and
    kx INSIDE the NEFF; marginal = (T(k) - T(1)) / (k-1) = pure HW time.
    Per-call dispatch overhead here is large (~60-90ms) and drifts, so T(1)
    and T(k) are sampled interleaved and the marginal is the median of the
    per-trial differences."""
    import time
    import jax
    from jax.sharding import NamedSharding, PartitionSpec

    C0, C1, table16, iota, ab, idx128, dval = preprocess(
        vals, mEmbed, inter, row_idx, col_idx)
    bk = dict(build_kwargs or {})
    per_core = [
        {"table": table16, "iota": iota, "ab": ab,
         "idx": idx128[k_], "dval": dval[k_]}
        for k_ in range(NCORES)
    ]

    def build(repeat):
        ck = (C0, C1, repeat, tuple(sorted(bk.items())))
        if ck not in _NC_CACHE:
            _NC_CACHE[ck] = build_nc(C0, C1, repeat=repeat, **bk)
        nc = _NC_CACHE[ck]
        sharded, mesh, in_names, out_names, zero_outs = _make_sharded(nc)
        sh = NamedSharding(mesh, PartitionSpec("core"))
        concat_in = [
            jax.device_put(
                np.concatenate([np.asarray(per_core[c][n]) for c in range(NCORES)],
                               axis=0), sh)
            for n in in_names
        ]
        concat_zero = [
            jax.device_put(np.zeros((NCORES * z.shape[0], *z.shape[1:]), z.dtype), sh)
            for z in zero_outs
        ]

        def run():
            out = sharded(*concat_in, *concat_zero)
            jax.block_until_ready(out)

        run()  # warm up executable + buffers
        return run

    run1 = build(1)
    runk = build(k)
    diffs, t1s, tks = [], [], []
    for _ in range(samples):
        t0 = time.perf_counter()
        run1()
        t1 = time.perf_counter()
        runk()
        t2 = time.perf_counter()
        t1s.append(t1 - t0)
        tks.append(t2 - t1)
        diffs.append(((t2 - t1) - (t1 - t0)) / (k - 1))
    diffs.sort()
    n = len(diffs)
    med = (diffs[(n - 1) // 2] + diffs[n // 2]) / 2
    return int(med * 1e9), int(min(t1s) * 1e9), int(min(tks) * 1e9)

